# revision 21
# baseline (speedup 1.0000x reference)
"""MoE layer (N=16384, D=1024, E=8, H=2048, top-2) on 8 trn2 NeuronCores.

Strategy: expert parallelism. The reference computes every expert densely but
only the top-2 survive the gather — so we dispatch each token to its two
routed experts only (4x compute saving). Core c owns expert c's weights; the
host computes the gating (bit-identically to the reference, CPU jax) and
all-to-all-dispatches gathered token batches; each core runs a dense
  y = (gelu(x @ W1 + b1) @ W2 + b2) * p
MLP over its batch with float32r matmuls (full PE rate, ~1e-4 precision);
the host scatter-adds the two expert contributions plus the residual.

Self-contained: only numpy/jax/concourse imports.
"""
import numpy as np

import concourse.bass as bass
import concourse.mybir as mybir
import concourse.tile as tile
from concourse.bass_utils import run_bass_kernel_spmd

N, D, E, H, TOP_K = 16384, 1024, 8, 2048, 2
P = 128
BGRAIN = 256     # capacity granularity; also the min/tail block width
BMAIN = 512      # main token block (moving dim per matmul)
KD = D // P      # 8 k-tiles over D
JH = H // P      # 16 h-tiles over H

TRACE = False          # test harness may flip this
LAST_RESULTS = None    # BassKernelResults of the last device run

F32 = mybir.dt.float32
F32R = mybir.dt.float32r


def _split_excess_waits(nc, max_waits=1):
    """This walrus build rejects >1 sem-wait per instruction; Tile emits more.
    Move excess waits onto same-engine NOPs inserted right before."""
    for fn in nc.m.functions:
        for blk in fn.blocks:
            insts = list(blk.instructions)
            out = []
            changed = False
            for inst in insts:
                si = getattr(inst, "sync_info", None)
                if si is not None and si.on_wait and len(si.on_wait) > max_waits:
                    waits = list(si.on_wait)
                    excess, keep = waits[:-max_waits], waits[-max_waits:]
                    for i in range(0, len(excess), max_waits):
                        out.append(
                            mybir.InstNoOp(
                                name=nc.get_next_instruction_name(),
                                engine=inst.engine,
                                sync_info=mybir.SyncInfo(
                                    on_wait=excess[i : i + max_waits], on_update=[]
                                ),
                                bass_nofuse=True,
                            )
                        )
                    inst.sync_info = mybir.SyncInfo(
                        on_wait=keep, on_update=list(si.on_update)
                    )
                    changed = True
                out.append(inst)
            if changed:
                blk.instructions = out


def _plan_blocks(C):
    """Split C into 512-wide blocks plus at most one 256 block (full-rate
    float32r needs moving dim >= 256). The narrow block goes FIRST: its
    smaller x DMA gets the PE off the ground sooner."""
    blocks, off = [], 0
    if C % BMAIN:
        blocks.append((0, C % BMAIN))
        off = C % BMAIN
    while C - off >= BMAIN:
        blocks.append((off, BMAIN))
        off += BMAIN
    return blocks


def build_nc(C: int):
    """Per-core dense expert MLP: yT = ((gelu(xT.T@w1+b1) @ w2) + b2).T * p."""
    nc = bass.Bass("TRN2", target_bir_lowering=False)
    xT = nc.dram_tensor("xT", (D, C), F32R, kind="ExternalInput")
    w1 = nc.dram_tensor("w1", (D, H), F32R, kind="ExternalInput")
    b1v = nc.dram_tensor("b1v", (H,), F32, kind="ExternalInput")
    w2 = nc.dram_tensor("w2", (H, D), F32R, kind="ExternalInput")
    b2v = nc.dram_tensor("b2v", (D,), F32, kind="ExternalInput")
    pv = nc.dram_tensor("pv", (P, C), F32, kind="ExternalInput")
    yT = nc.dram_tensor("yT", (D, C), F32, kind="ExternalOutput")

    xT_t = xT.rearrange("(k p) c -> p k c", p=P)
    yT_t = yT.rearrange("(k p) c -> p k c", p=P)

    with tile.TileContext(nc) as tc:
        with (
            tc.tile_pool(name="wpool", bufs=1) as wpool,
            tc.tile_pool(name="xpool", bufs=2) as xpool,
            tc.tile_pool(name="hpool", bufs=1) as hpool,
            tc.tile_pool(name="ypool", bufs=3) as ypool,
            tc.tile_pool(name="psum", bufs=3, space="PSUM") as psum,
        ):
            blocks = _plan_blocks(C)

            def load_block(off, B):
                xb = xpool.tile([P, KD, B], F32R, tag="xb")
                nc.sync.dma_start(xb[:], xT_t[:, :, off : off + B])
                pb = xpool.tile([P, B], F32, tag="pb")
                nc.sync.dma_start(pb[:], pv[:, off : off + B])
                return xb, pb

            # Hand-ordered DMA issue: the sync HWDGE queues carry the token
            # stream plus the earliest-needed weight slices (they start fast);
            # the gpsimd SWDGE queues carry the rest of the weights in
            # parallel. Per-output-tile weight slices mean a matmul chain only
            # waits for its own 0.5MB, not the whole 16MB.
            w1_t = w1.rearrange("(k p) h -> p k h", p=P)
            w2_t = w2.rearrange("(j p) d -> p j d", p=P)
            w1sb = [wpool.tile([P, KD, P], F32R, tag=f"w1_{j}", name=f"w1_{j}") for j in range(JH)]
            w2sb = [wpool.tile([P, JH, P], F32R, tag=f"w2_{d}", name=f"w2_{d}") for d in range(KD)]

            def load_w1(j, eng):
                eng.dma_start(w1sb[j][:], w1_t[:, :, j * P : (j + 1) * P])

            def load_w2(d, eng):
                eng.dma_start(w2sb[d][:], w2_t[:, :, d * P : (d + 1) * P])

            prefetched = [load_block(*blocks[0])]

            b1sb = wpool.tile([P, JH], F32)
            b2sb = wpool.tile([P, KD], F32)
            nc.gpsimd.dma_start(b1sb[:], b1v.rearrange("(o p) -> p o", p=P))
            nc.gpsimd.dma_start(b2sb[:], b2v.rearrange("(o p) -> p o", p=P))
            for j in range(4, JH):
                load_w1(j, nc.gpsimd)
            for d in (1, 3, 5, 7):
                load_w2(d, nc.gpsimd)

            for j in range(4):
                load_w1(j, nc.sync)
            if len(blocks) > 1:
                prefetched.append(load_block(*blocks[1]))
            for d in (0, 2):
                load_w2(d, nc.sync)

            for bi, (off, B) in enumerate(blocks):
                cs = slice(off, off + B)
                if bi < len(prefetched):
                    xb, pb = prefetched[bi]
                else:
                    xb, pb = load_block(off, B)
                if bi == 2:
                    load_w2(4, nc.sync)
                elif bi == 3:
                    load_w2(6, nc.sync)
                hb = hpool.tile([P, JH, B], F32R, tag="hb")
                # h^T[j] = gelu(W1[:, j].T @ x^T + b1[j])
                for j in range(JH):
                    ph = psum.tile([P, B], F32, tag="ph")
                    for k in range(KD):
                        nc.tensor.matmul(
                            ph[:],
                            w1sb[j][:, k],
                            xb[:, k],
                            start=(k == 0),
                            stop=(k == KD - 1),
                        )
                    nc.scalar.activation(
                        hb[:, j],
                        ph[:],
                        mybir.ActivationFunctionType.Gelu,
                        bias=b1sb[:, j : j + 1],
                    )
                # y^T[d] = (W2[:, d].T @ h^T + b2[d]) * p
                for d in range(KD):
                    pd = psum.tile([P, B], F32, tag="pd")
                    for j in range(JH):
                        nc.tensor.matmul(
                            pd[:],
                            w2sb[d][:, j],
                            hb[:, j],
                            start=(j == 0),
                            stop=(j == JH - 1),
                        )
                    yb = ypool.tile([P, B], F32, tag="yb")
                    nc.scalar.activation(
                        yb[:],
                        pd[:],
                        mybir.ActivationFunctionType.Identity,
                        bias=b2sb[:, d : d + 1],
                    )
                    nc.vector.tensor_mul(yb[:], yb[:], pb[:])
                    nc.sync.dma_start(yT_t[:, d, cs], yb[:])
    _split_excess_waits(nc)
    return nc


_NC_CACHE = {}


def _routing(x, Wg, bg):
    """Gating computed the same way (and on the same platform: CPU jax) as the
    reference, so the top-2 choice is bit-identical even for near-tie logits."""
    import jax
    import jax.numpy as jnp

    cpu = jax.local_devices(backend="cpu")[0]
    with jax.default_device(cpu):
        logits = jnp.asarray(x) @ jnp.asarray(Wg) + jnp.asarray(bg)
        probs = jax.nn.softmax(logits, axis=-1)
        topk_p, topk_i = jax.lax.top_k(probs, TOP_K)
        topk_p = topk_p / topk_p.sum(axis=-1, keepdims=True)
    return np.asarray(topk_i), np.asarray(topk_p)


def kernel(x, Wg, bg, W1, b1, W2, b2):
    global LAST_RESULTS
    x = np.ascontiguousarray(np.asarray(x, dtype=np.float32))
    Wg = np.asarray(Wg, dtype=np.float32)
    bg = np.asarray(bg, dtype=np.float32)
    W1 = np.asarray(W1, dtype=np.float32)
    b1 = np.asarray(b1, dtype=np.float32)
    W2 = np.asarray(W2, dtype=np.float32)
    b2 = np.asarray(b2, dtype=np.float32)

    topk_i, topk_p = _routing(x, Wg, bg)

    idx_list, p_list = [], []
    for e in range(E):
        m0 = topk_i[:, 0] == e
        m1 = topk_i[:, 1] == e
        idx = np.nonzero(m0 | m1)[0]
        p = np.where(m0[idx], topk_p[idx, 0], topk_p[idx, 1]).astype(np.float32)
        idx_list.append(idx)
        p_list.append(p)

    cmax = max(len(i) for i in idx_list)
    C = max(BGRAIN, ((cmax + BGRAIN - 1) // BGRAIN) * BGRAIN)

    if C not in _NC_CACHE:
        _NC_CACHE[C] = build_nc(C)
    nc = _NC_CACHE[C]

    in_maps = []
    for e in range(E):
        idx = idx_list[e]
        n = len(idx)
        xTg = np.zeros((D, C), np.float32)
        xTg[:, :n] = x[idx].T
        pvv = np.zeros((C,), np.float32)
        pvv[:n] = p_list[e]
        pvv = np.ascontiguousarray(np.broadcast_to(pvv, (P, C)))
        in_maps.append(
            {
                "xT": xTg,
                "w1": np.ascontiguousarray(W1[e]),
                "b1v": np.ascontiguousarray(b1[e]),
                "w2": np.ascontiguousarray(W2[e]),
                "b2v": np.ascontiguousarray(b2[e]),
                "pv": pvv,
            }
        )

    res = run_bass_kernel_spmd(nc, in_maps, core_ids=list(range(E)), trace=TRACE)
    LAST_RESULTS = res

    out = x.copy()
    for e in range(E):
        idx = idx_list[e]
        out[idx] += res.results[e]["yT"][:, : len(idx)].T
    return out
